# revision 8
# baseline (speedup 1.0000x reference)
"""Batched int8 GEMM (s8t x s8n -> s32t) on 8 TRN2 NeuronCores.

out[b, m, n] = sum_k a[b, m, k] * b[b, n, k]   (int32 accumulation)
a: [32, 1024, 1024] int8, b: [32, 1024, 1024] int8 -> out: [32, 1024, 1024] int32

Strategy:
  - Pure batch parallelism: 4 batches per core across 8 cores.
  - Both operands have K innermost, but the PE needs K on partitions.
    DMA-transpose works on 2-byte elements only, so we view the int8
    inputs as uint16 (pairs of adjacent K values) and DMA-transpose
    per-batch K-blocks of 256 K-values ([1024, 128] uint16 ->
    [128, 1024]), each partition holding an even/odd K pair interleaved
    along the free dim. Per-batch (rather than batch-pair) transposes
    keep each transpose's consumers within one batch so its DMA
    semaphore lane recycles quickly (8 HWDGE lanes rotate over all
    HWDGE DMAs; a lane is not reusable until the prior user's consumers
    have executed).
  - DVE deinterleaves (stride-2 int8 reads) and converts int8 -> bf16.
    int8 is exactly representable in bf16; products <= 2^14 and sums
    <= 2^24 are exact in fp32 PSUM accumulation, so the GEMM is
    bit-exact (native int8 matmul is rejected by walrus's BIR verifier,
    so bf16 is the fastest exact path; fp8 DoubleRow needs a 3-product
    Karatsuba digit split = 1.5x the PE cycles of bf16, a net loss).
  - PE: bf16 matmuls, K=128 per instruction, 8-step accumulation into
    [128, 512] fp32 PSUM banks (8 banks in flight). 16 dummy matmuls up
    front warm the HAM clock gate.
  - PSUM-freeing copies (fp32 PSUM -> int32 SBUF, exact) run on ACT
    (GPSIMD cannot access PSUM; DVE is busy with the deint stream).
    The final mt block's second copy goes on the by-then-idle DVE.
  - Stores: batches 0-1 issue one deferred 4MiB store each on ACT
    (deferred into the middle of the next batch's copy stream so the
    store's semaphore-lane WAR wait resolves during ACT idle time).
    Batch 2's 4MiB store is issued from SYNC (idle once all transposes
    are done) the moment batch 2's copies complete (~91us), so the
    output DMA queue is fully drained before the last batch's stores
    arrive. The last batch stores per-mt from SYNC (prompt dispatch
    from an otherwise-empty FIFO; on ACT they dispatch up to 2.2us
    late behind the next copy's semaphore wait), and the final mt
    block is split into two 256KB per-nt stores issued in parallel
    from ACT and SYNC so the kernel tail only waits on ~256KB.
"""

import numpy as np

import concourse.bass as bass
import concourse.mybir as mybir
import concourse.tile as tile
from concourse import bacc
from concourse.bass_utils import run_bass_kernel_spmd
from concourse.tile_rust import add_dep_helper

B, M, N, K = 32, 1024, 1024, 1024
N_CORES = 8
BPC = B // N_CORES  # batches per core
KB = K // 256  # k-blocks of 256 K-values (128 uint16 partitions)
N_TILE = 512
M_TILE = 128

_nc_cache = None


def build_nc():
    nc = bacc.Bacc("TRN2")

    # int8 inputs viewed as uint16 so the xbar DMA-transpose (2-byte
    # granularity) can be used straight out of HBM.
    a_in = nc.dram_tensor("a", [BPC, M, K // 2], mybir.dt.uint16, kind="ExternalInput")
    b_in = nc.dram_tensor("b", [BPC, N, K // 2], mybir.dt.uint16, kind="ExternalInput")
    out = nc.dram_tensor("out", [BPC, M, N], mybir.dt.int32, kind="ExternalOutput")

    with tile.TileContext(nc) as tc:
        with (
            tc.tile_pool(name="stage", bufs=2) as stage_pool,
            tc.tile_pool(name="conv", bufs=2) as conv_pool,
            tc.tile_pool(name="psum", bufs=8, space="PSUM") as psum_pool,
            tc.tile_pool(name="outbuf", bufs=2) as out_pool,
            tc.tile_pool(name="warm", bufs=1) as warm_pool,
            tc.tile_pool(name="stage_h", bufs=1) as stageh_pool,
            tc.tile_pool(name="conv_h", bufs=1) as convh_pool,
        ):
            # PE warmup: dummy matmuls with NO deps at all (uninitialized
            # SBUF reads are fine; the PSUM result is discarded), so the
            # HAM clock gate ramps before the real MM stream starts. Ten
            # warmups bridge the ~3.5us between the PE becoming ready
            # (~7.5us) and the first deinted tiles arriving (~11us);
            # alternating two PSUM banks lets consecutive start/stop
            # matmuls pipeline once the clock is hot.
            wsrc = warm_pool.tile([128, N_TILE], mybir.dt.bfloat16, name="wsrc")
            nc.vector.memset(wsrc[:, :8], 0.0)
            wps = [
                psum_pool.tile([128, N_TILE], mybir.dt.float32, name=f"wps{i}", tag="ps")
                for i in range(2)
            ]
            for i in range(10):
                nc.tensor.matmul(
                    wps[i % 2][:], wsrc[:, :128], wsrc[:], start=True, stop=True
                )

            # Batches 0-1: stores are deferred into the middle of the NEXT
            # batch's copy stream (see docstring). Batch 2's store goes on
            # SYNC as soon as its data is ready; the last batch stores
            # per-mt on SYNC.
            pending_store = None
            ot_by_batch = {}
            HKB = 2  # kb blocks staged as M/N halves for batch 0
            for bi in range(BPC):
                n_kt = 2 * KB
                n_mt = M // M_TILE
                if bi == 0:
                    # ---- Batch 0: latency-critical prologue. kb0/kb1 are
                    # transposed as [512, 128] M/N-halves so the first
                    # deinted tiles arrive ~2us sooner than with full
                    # 256KB blocks; a's M-upper halves are only needed by
                    # the second mt-group (~25us in) and transpose last. ----
                    h_st = {}

                    def h_transpose(t, kb, h):
                        src = a_in if t == "a" else b_in
                        ht = stageh_pool.tile(
                            [128, 512],
                            mybir.dt.uint16,
                            name=f"h{t}{kb}{h}",
                            tag=f"h{t}{kb}{h}",
                        )
                        nc.sync.dma_start_transpose(
                            ht[:],
                            src[bi, h * 512 : (h + 1) * 512, kb * 128 : (kb + 1) * 128],
                        )
                        h_st[(t, kb, h)] = ht.bitcast(mybir.dt.int8)

                    for kb in range(HKB):
                        h_transpose("a", kb, 0)
                        h_transpose("b", kb, 0)
                        h_transpose("b", kb, 1)
                    a_st = {}
                    b_st = {}
                    for kb in range(HKB, KB):
                        at = stage_pool.tile(
                            [128, M], mybir.dt.uint16, name=f"at_{bi}_{kb}", tag=f"at{kb}"
                        )
                        nc.sync.dma_start_transpose(
                            at[:], a_in[bi, :, kb * 128 : (kb + 1) * 128]
                        )
                        a_st[kb] = at.bitcast(mybir.dt.int8)
                        bt = stage_pool.tile(
                            [128, N], mybir.dt.uint16, name=f"bt_{bi}_{kb}", tag=f"bt{kb}"
                        )
                        nc.sync.dma_start_transpose(
                            bt[:], b_in[bi, :, kb * 128 : (kb + 1) * 128]
                        )
                        b_st[kb] = bt.bitcast(mybir.dt.int8)
                    for kb in range(HKB):
                        h_transpose("a", kb, 1)

                    # ---- deints (DVE) in kt-consumption order: per kt the
                    # g0 stream needs a's M-lower half plus both b halves. ----
                    h_bf = {}

                    def h_deint(t, kb, h, par):
                        hb = convh_pool.tile(
                            [128, 512],
                            mybir.dt.bfloat16,
                            name=f"hb{t}{kb}{h}{par}",
                            tag=f"hb{t}{kb}{h}{par}",
                        )
                        nc.vector.tensor_copy(hb[:], h_st[(t, kb, h)][:, par::2])
                        h_bf[(t, kb, h, par)] = hb

                    a_bf = {}
                    b_bf = {}
                    for kb in range(HKB):
                        for par in range(2):
                            h_deint("a", kb, 0, par)
                            h_deint("b", kb, 0, par)
                            h_deint("b", kb, 1, par)
                    for kb in range(HKB, KB):
                        for par in range(2):
                            abf = conv_pool.tile(
                                [128, M],
                                mybir.dt.bfloat16,
                                name=f"abf_{bi}_{kb}_{par}",
                                tag=f"abf{kb}{par}",
                            )
                            nc.vector.tensor_copy(abf[:], a_st[kb][:, par::2])
                            a_bf[2 * kb + par] = abf
                            bbf = conv_pool.tile(
                                [128, N],
                                mybir.dt.bfloat16,
                                name=f"bbf_{bi}_{kb}_{par}",
                                tag=f"bbf{kb}{par}",
                            )
                            nc.vector.tensor_copy(bbf[:], b_st[kb][:, par::2])
                            b_bf[2 * kb + par] = bbf
                    for kb in range(HKB):
                        for par in range(2):
                            h_deint("a", kb, 1, par)

                    def a_sl(g, kt, mt):
                        kb, par = kt // 2, kt % 2
                        if kb < HKB:
                            return h_bf[("a", kb, g, par)][
                                :, (mt - 4 * g) * M_TILE : (mt - 4 * g + 1) * M_TILE
                            ]
                        return a_bf[kt][:, mt * M_TILE : (mt + 1) * M_TILE]

                    def b_sl(kt, nt):
                        kb, par = kt // 2, kt % 2
                        if kb < HKB:
                            return h_bf[("b", kb, nt, par)][:]
                        return b_bf[kt][:, nt * N_TILE : (nt + 1) * N_TILE]

                    ot = out_pool.tile(
                        [128, n_mt, N], mybir.dt.int32, name=f"ot_{bi}", tag="ot"
                    )
                    ot_by_batch[bi] = ot
                    # kt-outer over groups of 4 mt blocks (8 PSUM banks) so
                    # each arriving k-tile feeds 1.73us of real PE work; nt
                    # before mt inside a kt because b's N-upper half arrives
                    # ~0.35us after the N-lower half.
                    for g in range(n_mt // 4):
                        mts = range(4 * g, 4 * g + 4)
                        ps = {
                            (mt, nt): psum_pool.tile(
                                [128, N_TILE],
                                mybir.dt.float32,
                                name=f"ps_{bi}_{mt}_{nt}",
                                tag="ps",
                            )
                            for mt in mts
                            for nt in range(N // N_TILE)
                        }
                        for kt in range(n_kt):
                            for nt in range(N // N_TILE):
                                for mt in mts:
                                    nc.tensor.matmul(
                                        ps[(mt, nt)][:],
                                        a_sl(g, kt, mt),
                                        b_sl(kt, nt),
                                        start=(kt == 0),
                                        stop=(kt == n_kt - 1),
                                    )
                        for mt in mts:
                            for nt in range(N // N_TILE):
                                nc.scalar.copy(
                                    ot[:, mt, nt * N_TILE : (nt + 1) * N_TILE],
                                    ps[(mt, nt)][:],
                                )
                    pending_store = (bi, ot)
                    continue

                # ---- Batches 1-3: per-batch DMA-transpose staging: each
                # transpose's consumers (2 deints) execute within this
                # batch's prologue, so its semaphore lane recycles quickly. ----
                a_st = []
                b_st = []
                for kb in range(KB):
                    at = stage_pool.tile(
                        [128, M], mybir.dt.uint16, name=f"at_{bi}_{kb}", tag=f"at{kb}"
                    )
                    nc.sync.dma_start_transpose(at[:], a_in[bi, :, kb * 128 : (kb + 1) * 128])
                    a_st.append(at.bitcast(mybir.dt.int8))
                    bt = stage_pool.tile(
                        [128, N], mybir.dt.uint16, name=f"bt_{bi}_{kb}", tag=f"bt{kb}"
                    )
                    nc.sync.dma_start_transpose(bt[:], b_in[bi, :, kb * 128 : (kb + 1) * 128])
                    b_st.append(bt.bitcast(mybir.dt.int8))

                if bi == BPC - 1:
                    # Batch 2's 4MiB store, emitted right after the last
                    # batch's transposes: SYNC's FIFO is otherwise empty
                    # from ~60us on, so this dispatches the moment batch
                    # 2's copies complete (~91us) and the output queue is
                    # drained well before the last batch's stores arrive.
                    pbi = BPC - 2
                    nc.sync.dma_start(
                        out[pbi].rearrange("(t p) n -> p t n", p=128),
                        ot_by_batch[pbi][:],
                    )

                # ---- deinterleave + int8 -> bf16 (DVE) ----
                a_bf = []  # 8 bf16 tiles [128, M]; k-tile = kb*2+parity
                b_bf = []
                for kb in range(KB):
                    for par in range(2):
                        abf = conv_pool.tile(
                            [128, M],
                            mybir.dt.bfloat16,
                            name=f"abf_{bi}_{kb}_{par}",
                            tag=f"abf{kb}{par}",
                        )
                        nc.vector.tensor_copy(abf[:], a_st[kb][:, par::2])
                        a_bf.append(abf)
                        bbf = conv_pool.tile(
                            [128, N],
                            mybir.dt.bfloat16,
                            name=f"bbf_{bi}_{kb}_{par}",
                            tag=f"bbf{kb}{par}",
                        )
                        nc.vector.tensor_copy(bbf[:], b_st[kb][:, par::2])
                        b_bf.append(bbf)

                # ---- GEMM, accumulating in PSUM over kt. All 8 mt blocks
                # copy into one big staging tile. ----
                ot = out_pool.tile(
                    [128, n_mt, N], mybir.dt.int32, name=f"ot_{bi}", tag="ot"
                )
                ot_by_batch[bi] = ot
                if True:
                    # Steady-state batches: mt-outer so the PSUM-freeing
                    # copies spread evenly instead of bunching.
                    for mt in range(n_mt):
                        ps = [
                            psum_pool.tile(
                                [128, N_TILE],
                                mybir.dt.float32,
                                name=f"ps_{bi}_{mt}_{nt}",
                                tag="ps",
                            )
                            for nt in range(N // N_TILE)
                        ]
                        for kt in range(n_kt):
                            lhsT = a_bf[kt][:, mt * M_TILE : (mt + 1) * M_TILE]
                            for nt in range(N // N_TILE):
                                nc.tensor.matmul(
                                    ps[nt][:],
                                    lhsT,
                                    b_bf[kt][:, nt * N_TILE : (nt + 1) * N_TILE],
                                    start=(kt == 0),
                                    stop=(kt == n_kt - 1),
                                )
                        # fp32 -> int32 PSUM-freeing copies on ACT (exact:
                        # values are integers). For the very last mt block,
                        # the second copy goes on the (by then idle) DVE so
                        # the two copies run in parallel and the final
                        # stores start ~0.7us sooner.
                        if bi == BPC - 1 and mt == n_mt - 1:
                            act_copy = nc.scalar.copy(ot[:, mt, :N_TILE], ps[0][:])
                            nc.vector.tensor_copy(ot[:, mt, N_TILE:], ps[1][:])
                        else:
                            act_copy = None
                            for nt in range(N // N_TILE):
                                act_copy = nc.scalar.copy(
                                    ot[:, mt, nt * N_TILE : (nt + 1) * N_TILE], ps[nt][:]
                                )
                        if mt == 3 and pending_store is not None:
                            pbi, pot = pending_store
                            st = nc.scalar.dma_start(
                                out[pbi].rearrange("(t p) n -> p t n", p=128), pot[:]
                            )
                            # Ordering-only edge: keep the store (and its
                            # semaphore-lane WAR wait) behind this batch's
                            # mt0-3 ACT copies in the ACT FIFO.
                            add_dep_helper(
                                st.ins,
                                act_copy.ins,
                                False,
                                "defer batch store past next batch's early copies",
                            )
                            pending_store = None
                        if bi == BPC - 1:
                            if mt == n_mt - 1:
                                # Final mt: two 256KB per-nt stores issued in
                                # parallel from ACT and SYNC so the kernel
                                # tail only waits on the last ~256KB.
                                nc.scalar.dma_start(
                                    out[bi, mt * M_TILE : (mt + 1) * M_TILE, :N_TILE],
                                    ot[:, mt, :N_TILE],
                                )
                                nc.sync.dma_start(
                                    out[bi, mt * M_TILE : (mt + 1) * M_TILE, N_TILE:],
                                    ot[:, mt, N_TILE:],
                                )
                            else:
                                # Last batch: store per mt from SYNC (prompt
                                # dispatch; on ACT these sit up to 2.2us
                                # behind the next copy's semaphore wait).
                                nc.sync.dma_start(
                                    out[bi, mt * M_TILE : (mt + 1) * M_TILE, :],
                                    ot[:, mt, :],
                                )
                if bi < BPC - 2:
                    # Batches 0-1: one 4MiB store for the whole batch,
                    # deferred (emitted mid-next-batch, see above). HBM row
                    # (mt*128 + p) pairs with SBUF [p, mt, :].
                    pending_store = (bi, ot)
    nc.compile()
    return nc


def _get_nc():
    global _nc_cache
    if _nc_cache is None:
        _nc_cache = build_nc()
    return _nc_cache


def run(a: np.ndarray, b: np.ndarray, trace: bool = False):
    """Run on 8 cores. a/b: [32, 1024, 1024] int8. Returns (out, BassKernelResults)."""
    a = np.ascontiguousarray(a)
    b = np.ascontiguousarray(b)
    a16 = a.view(np.uint16).reshape(B, M, K // 2)
    b16 = b.view(np.uint16).reshape(B, N, K // 2)
    in_maps = [
        {
            "a": a16[c * BPC : (c + 1) * BPC],
            "b": b16[c * BPC : (c + 1) * BPC],
        }
        for c in range(N_CORES)
    ]
    res = run_bass_kernel_spmd(_get_nc(), in_maps, list(range(N_CORES)), trace=trace)
    out = np.concatenate([res.results[c]["out"] for c in range(N_CORES)], axis=0)
    return out, res


def kernel(a: np.ndarray, b: np.ndarray) -> np.ndarray:
    out, _ = run(np.asarray(a), np.asarray(b))
    return out


# revision 14
# speedup vs baseline: 1.0008x; 1.0008x over previous
"""Batched int8 GEMM (s8t x s8n -> s32t) on 8 TRN2 NeuronCores.

out[b, m, n] = sum_k a[b, m, k] * b[b, n, k]   (int32 accumulation)
a: [32, 1024, 1024] int8, b: [32, 1024, 1024] int8 -> out: [32, 1024, 1024] int32

Strategy:
  - Pure batch parallelism: 4 batches per core across 8 cores.
  - Both operands have K innermost, but the PE needs K on partitions.
    DMA-transpose works on 2-byte elements only, so we view the int8
    inputs as uint16 (pairs of adjacent K values) and DMA-transpose
    per-batch K-blocks of 256 K-values ([1024, 128] uint16 ->
    [128, 1024]), each partition holding an even/odd K pair interleaved
    along the free dim. Per-batch (rather than batch-pair) transposes
    keep each transpose's consumers within one batch so its DMA
    semaphore lane recycles quickly (8 HWDGE lanes rotate over all
    HWDGE DMAs; a lane is not reusable until the prior user's consumers
    have executed).
  - DVE deinterleaves (stride-2 int8 reads) and converts int8 -> bf16.
    int8 is exactly representable in bf16; products <= 2^14 and sums
    <= 2^24 are exact in fp32 PSUM accumulation, so the GEMM is
    bit-exact (native int8 matmul is rejected by walrus's BIR verifier,
    so bf16 is the fastest exact path; fp8 DoubleRow needs a 3-product
    Karatsuba digit split = 1.5x the PE cycles of bf16, a net loss).
  - PE: bf16 matmuls, K=128 per instruction, 8-step accumulation into
    [128, 512] fp32 PSUM banks (8 banks in flight). 16 dummy matmuls up
    front warm the HAM clock gate.
  - PSUM-freeing copies (fp32 PSUM -> int32 SBUF, exact) run on ACT
    (GPSIMD cannot access PSUM; DVE is busy with the deint stream).
    The final mt block's second copy goes on the by-then-idle DVE.
  - Stores: batches 0-1 issue one deferred 4MiB store each on ACT
    (deferred into the middle of the next batch's copy stream so the
    store's semaphore-lane WAR wait resolves during ACT idle time).
    Batch 2's 4MiB store is issued from SYNC (idle once all transposes
    are done) the moment batch 2's copies complete (~91us), so the
    output DMA queue is fully drained before the last batch's stores
    arrive. The last batch stores per-mt from SYNC (prompt dispatch
    from an otherwise-empty FIFO; on ACT they dispatch up to 2.2us
    late behind the next copy's semaphore wait), and the final mt
    block is split into two 256KB per-nt stores issued in parallel
    from ACT and SYNC so the kernel tail only waits on ~256KB.
"""

import numpy as np

import concourse.bass as bass
import concourse.mybir as mybir
import concourse.tile as tile
from concourse import bacc
from concourse.bass_utils import run_bass_kernel_spmd
from concourse.tile_rust import add_dep_helper

B, M, N, K = 32, 1024, 1024, 1024
N_CORES = 8
BPC = B // N_CORES  # batches per core
KB = K // 256  # k-blocks of 256 K-values (128 uint16 partitions)
N_TILE = 512
M_TILE = 128

_nc_cache = None


def build_nc():
    nc = bacc.Bacc("TRN2")

    # int8 inputs viewed as uint16 so the xbar DMA-transpose (2-byte
    # granularity) can be used straight out of HBM.
    a_in = nc.dram_tensor("a", [BPC, M, K // 2], mybir.dt.uint16, kind="ExternalInput")
    b_in = nc.dram_tensor("b", [BPC, N, K // 2], mybir.dt.uint16, kind="ExternalInput")
    out = nc.dram_tensor("out", [BPC, M, N], mybir.dt.int32, kind="ExternalOutput")

    with tile.TileContext(nc) as tc:
        with (
            tc.tile_pool(name="stage", bufs=2) as stage_pool,
            tc.tile_pool(name="conv", bufs=2) as conv_pool,
            tc.tile_pool(name="psum", bufs=8, space="PSUM") as psum_pool,
            tc.tile_pool(name="outbuf", bufs=2) as out_pool,
            tc.tile_pool(name="warm", bufs=1) as warm_pool,
            tc.tile_pool(name="stage_h", bufs=1) as stageh_pool,
            tc.tile_pool(name="conv_h", bufs=1) as convh_pool,
        ):
            # PE warmup: dummy matmuls with NO deps at all (uninitialized
            # SBUF reads are fine; the PSUM result is discarded), so the
            # HAM clock gate ramps before the real MM stream starts. Ten
            # warmups bridge the ~3.5us between the PE becoming ready
            # (~7.5us) and the first deinted tiles arriving (~11us);
            # alternating two PSUM banks lets consecutive start/stop
            # matmuls pipeline once the clock is hot.
            wsrc = warm_pool.tile([128, N_TILE], mybir.dt.bfloat16, name="wsrc")
            nc.vector.memset(wsrc[:, :8], 0.0)
            wps = [
                psum_pool.tile([128, N_TILE], mybir.dt.float32, name=f"wps{i}", tag="ps")
                for i in range(2)
            ]
            for i in range(9):
                nc.tensor.matmul(
                    wps[i % 2][:], wsrc[:, :128], wsrc[:], start=True, stop=True
                )

            # Batches 0-1: stores are deferred into the middle of the NEXT
            # batch's copy stream (see docstring). Batch 2's store goes on
            # SYNC as soon as its data is ready; the last batch stores
            # per-mt on SYNC.
            pending_store = None
            ot_by_batch = {}
            HKB = 1  # kb blocks staged as M/N halves for batch 0
            for bi in range(BPC):
                n_kt = 2 * KB
                n_mt = M // M_TILE
                if bi == 0:
                    # ---- Batch 0: latency-critical prologue. Transpose
                    # desc-gen costs ~1.3us per instruction almost
                    # regardless of size (all on SYNC: issuing from two
                    # engines concurrently corrupts data - the 8 HWDGE
                    # semaphore lanes race across engines). kb0 is staged
                    # as [512, 128] M/N-halves so the first deinted tiles
                    # arrive ~1.5us sooner; a's M-upper half is only needed
                    # by the second mt-group (~25us in) and transposes
                    # last. ----
                    h_st = {}

                    def h_transpose(t, kb, h):
                        src = a_in if t == "a" else b_in
                        ht = stageh_pool.tile(
                            [128, 512],
                            mybir.dt.uint16,
                            name=f"h{t}{kb}{h}",
                            tag=f"h{t}{kb}{h}",
                        )
                        nc.sync.dma_start_transpose(
                            ht[:],
                            src[bi, h * 512 : (h + 1) * 512, kb * 128 : (kb + 1) * 128],
                        )
                        h_st[(t, kb, h)] = ht.bitcast(mybir.dt.int8)

                    for kb in range(HKB):
                        h_transpose("a", kb, 0)
                        h_transpose("b", kb, 0)
                        h_transpose("b", kb, 1)
                    a_st = {}
                    b_st = {}
                    for kb in range(HKB, KB):
                        at = stage_pool.tile(
                            [128, M], mybir.dt.uint16, name=f"at_{bi}_{kb}", tag=f"at{kb}"
                        )
                        nc.sync.dma_start_transpose(
                            at[:], a_in[bi, :, kb * 128 : (kb + 1) * 128]
                        )
                        a_st[kb] = at.bitcast(mybir.dt.int8)
                        bt = stage_pool.tile(
                            [128, N], mybir.dt.uint16, name=f"bt_{bi}_{kb}", tag=f"bt{kb}"
                        )
                        nc.sync.dma_start_transpose(
                            bt[:], b_in[bi, :, kb * 128 : (kb + 1) * 128]
                        )
                        b_st[kb] = bt.bitcast(mybir.dt.int8)
                    for kb in range(HKB):
                        h_transpose("a", kb, 1)

                    # ---- deints (DVE) in kt-consumption order: per kt the
                    # g0 stream needs a's M-lower half plus b's full N. ----
                    h_bf = {}

                    def h_deint(t, kb, h, par):
                        hb = convh_pool.tile(
                            [128, 512],
                            mybir.dt.bfloat16,
                            name=f"hb{t}{kb}{h}{par}",
                            tag=f"hb{t}{kb}{h}{par}",
                        )
                        nc.vector.tensor_copy(hb[:], h_st[(t, kb, h)][:, par::2])
                        h_bf[(t, kb, h, par)] = hb

                    a_bf = {}
                    b_bf = {}
                    for kb in range(HKB):
                        for par in range(2):
                            h_deint("a", kb, 0, par)
                            h_deint("b", kb, 0, par)
                            h_deint("b", kb, 1, par)
                    for kb in range(HKB, KB):
                        for par in range(2):
                            abf = conv_pool.tile(
                                [128, M],
                                mybir.dt.bfloat16,
                                name=f"abf_{bi}_{kb}_{par}",
                                tag=f"abf{kb}{par}",
                            )
                            nc.vector.tensor_copy(abf[:], a_st[kb][:, par::2])
                            a_bf[2 * kb + par] = abf
                            bbf = conv_pool.tile(
                                [128, N],
                                mybir.dt.bfloat16,
                                name=f"bbf_{bi}_{kb}_{par}",
                                tag=f"bbf{kb}{par}",
                            )
                            nc.vector.tensor_copy(bbf[:], b_st[kb][:, par::2])
                            b_bf[2 * kb + par] = bbf
                    for kb in range(HKB):
                        for par in range(2):
                            h_deint("a", kb, 1, par)

                    def a_sl(g, kt, mt):
                        kb, par = kt // 2, kt % 2
                        if kb < HKB:
                            return h_bf[("a", kb, g, par)][
                                :, (mt - 4 * g) * M_TILE : (mt - 4 * g + 1) * M_TILE
                            ]
                        return a_bf[kt][:, mt * M_TILE : (mt + 1) * M_TILE]

                    def b_sl(kt, nt):
                        kb, par = kt // 2, kt % 2
                        if kb < HKB:
                            return h_bf[("b", kb, nt, par)][:]
                        return b_bf[kt][:, nt * N_TILE : (nt + 1) * N_TILE]

                    ot = out_pool.tile(
                        [128, n_mt, N], mybir.dt.int32, name=f"ot_{bi}", tag="ot"
                    )
                    ot_by_batch[bi] = ot
                    # kt-outer over groups of 4 mt blocks (8 PSUM banks) so
                    # each arriving k-tile feeds 1.73us of real PE work; nt
                    # before mt inside a kt because b's N-upper half arrives
                    # ~0.35us after the N-lower half.
                    for g in range(n_mt // 4):
                        mts = range(4 * g, 4 * g + 4)
                        ps = {
                            (mt, nt): psum_pool.tile(
                                [128, N_TILE],
                                mybir.dt.float32,
                                name=f"ps_{bi}_{mt}_{nt}",
                                tag="ps",
                            )
                            for mt in mts
                            for nt in range(N // N_TILE)
                        }
                        for kt in range(n_kt):
                            for nt in range(N // N_TILE):
                                for mt in mts:
                                    nc.tensor.matmul(
                                        ps[(mt, nt)][:],
                                        a_sl(g, kt, mt),
                                        b_sl(kt, nt),
                                        start=(kt == 0),
                                        stop=(kt == n_kt - 1),
                                    )
                        for mt in mts:
                            for nt in range(N // N_TILE):
                                nc.scalar.copy(
                                    ot[:, mt, nt * N_TILE : (nt + 1) * N_TILE],
                                    ps[(mt, nt)][:],
                                )
                    pending_store = (bi, ot)
                    continue

                # ---- Batches 1-3: per-batch DMA-transpose staging: each
                # transpose's consumers (2 deints) execute within this
                # batch's prologue, so its semaphore lane recycles quickly. ----
                a_st = []
                b_st = []
                for kb in range(KB):
                    at = stage_pool.tile(
                        [128, M], mybir.dt.uint16, name=f"at_{bi}_{kb}", tag=f"at{kb}"
                    )
                    nc.sync.dma_start_transpose(at[:], a_in[bi, :, kb * 128 : (kb + 1) * 128])
                    a_st.append(at.bitcast(mybir.dt.int8))
                    bt = stage_pool.tile(
                        [128, N], mybir.dt.uint16, name=f"bt_{bi}_{kb}", tag=f"bt{kb}"
                    )
                    nc.sync.dma_start_transpose(bt[:], b_in[bi, :, kb * 128 : (kb + 1) * 128])
                    b_st.append(bt.bitcast(mybir.dt.int8))

                if bi == BPC - 1:
                    # Batch 2's 4MiB store, emitted right after the last
                    # batch's transposes: SYNC's FIFO is otherwise empty
                    # from ~60us on, so this dispatches the moment batch
                    # 2's copies complete (~91us) and the output queue is
                    # drained well before the last batch's stores arrive.
                    pbi = BPC - 2
                    nc.sync.dma_start(
                        out[pbi].rearrange("(t p) n -> p t n", p=128),
                        ot_by_batch[pbi][:],
                    )

                # ---- deinterleave + int8 -> bf16 (DVE) ----
                a_bf = []  # 8 bf16 tiles [128, M]; k-tile = kb*2+parity
                b_bf = []
                for kb in range(KB):
                    for par in range(2):
                        abf = conv_pool.tile(
                            [128, M],
                            mybir.dt.bfloat16,
                            name=f"abf_{bi}_{kb}_{par}",
                            tag=f"abf{kb}{par}",
                        )
                        nc.vector.tensor_copy(abf[:], a_st[kb][:, par::2])
                        a_bf.append(abf)
                        bbf = conv_pool.tile(
                            [128, N],
                            mybir.dt.bfloat16,
                            name=f"bbf_{bi}_{kb}_{par}",
                            tag=f"bbf{kb}{par}",
                        )
                        nc.vector.tensor_copy(bbf[:], b_st[kb][:, par::2])
                        b_bf.append(bbf)

                # ---- GEMM, accumulating in PSUM over kt. All 8 mt blocks
                # copy into one big staging tile. ----
                ot = out_pool.tile(
                    [128, n_mt, N], mybir.dt.int32, name=f"ot_{bi}", tag="ot"
                )
                ot_by_batch[bi] = ot
                if True:
                    # Steady-state batches: mt-outer so the PSUM-freeing
                    # copies spread evenly instead of bunching.
                    for mt in range(n_mt):
                        ps = [
                            psum_pool.tile(
                                [128, N_TILE],
                                mybir.dt.float32,
                                name=f"ps_{bi}_{mt}_{nt}",
                                tag="ps",
                            )
                            for nt in range(N // N_TILE)
                        ]
                        for kt in range(n_kt):
                            lhsT = a_bf[kt][:, mt * M_TILE : (mt + 1) * M_TILE]
                            for nt in range(N // N_TILE):
                                nc.tensor.matmul(
                                    ps[nt][:],
                                    lhsT,
                                    b_bf[kt][:, nt * N_TILE : (nt + 1) * N_TILE],
                                    start=(kt == 0),
                                    stop=(kt == n_kt - 1),
                                )
                        # fp32 -> int32 PSUM-freeing copies on ACT (exact:
                        # values are integers). For the very last mt block,
                        # the second copy goes on the (by then idle) DVE so
                        # the two copies run in parallel and the final
                        # stores start ~0.7us sooner.
                        if bi == BPC - 1 and mt == n_mt - 1:
                            act_copy = nc.scalar.copy(ot[:, mt, :N_TILE], ps[0][:])
                            nc.vector.tensor_copy(ot[:, mt, N_TILE:], ps[1][:])
                        else:
                            act_copy = None
                            for nt in range(N // N_TILE):
                                act_copy = nc.scalar.copy(
                                    ot[:, mt, nt * N_TILE : (nt + 1) * N_TILE], ps[nt][:]
                                )
                        if mt == 3 and pending_store is not None:
                            pbi, pot = pending_store
                            st = nc.scalar.dma_start(
                                out[pbi].rearrange("(t p) n -> p t n", p=128), pot[:]
                            )
                            # Ordering-only edge: keep the store (and its
                            # semaphore-lane WAR wait) behind this batch's
                            # mt0-3 ACT copies in the ACT FIFO.
                            add_dep_helper(
                                st.ins,
                                act_copy.ins,
                                False,
                                "defer batch store past next batch's early copies",
                            )
                            pending_store = None
                        if bi == BPC - 1:
                            if mt == n_mt - 1:
                                # Final mt: four 128KB quarter stores, two
                                # per HWDGE queue (ACT->q10, SYNC->q1), so
                                # the kernel tail only waits on the last
                                # ~128KB to drain.
                                Q = N_TILE // 2
                                rows = slice(mt * M_TILE, (mt + 1) * M_TILE)
                                for q in range(2):
                                    nc.scalar.dma_start(
                                        out[bi, rows, q * Q : (q + 1) * Q],
                                        ot[:, mt, q * Q : (q + 1) * Q],
                                    )
                                for q in range(2, 4):
                                    nc.sync.dma_start(
                                        out[bi, rows, q * Q : (q + 1) * Q],
                                        ot[:, mt, q * Q : (q + 1) * Q],
                                    )
                            else:
                                # Last batch: store per mt from SYNC (prompt
                                # dispatch; on ACT these sit up to 2.2us
                                # behind the next copy's semaphore wait).
                                nc.sync.dma_start(
                                    out[bi, mt * M_TILE : (mt + 1) * M_TILE, :],
                                    ot[:, mt, :],
                                )
                if bi < BPC - 2:
                    # Batches 0-1: one 4MiB store for the whole batch,
                    # deferred (emitted mid-next-batch, see above). HBM row
                    # (mt*128 + p) pairs with SBUF [p, mt, :].
                    pending_store = (bi, ot)
    nc.compile()
    return nc


def _get_nc():
    global _nc_cache
    if _nc_cache is None:
        _nc_cache = build_nc()
    return _nc_cache


def run(a: np.ndarray, b: np.ndarray, trace: bool = False):
    """Run on 8 cores. a/b: [32, 1024, 1024] int8. Returns (out, BassKernelResults)."""
    a = np.ascontiguousarray(a)
    b = np.ascontiguousarray(b)
    a16 = a.view(np.uint16).reshape(B, M, K // 2)
    b16 = b.view(np.uint16).reshape(B, N, K // 2)
    in_maps = [
        {
            "a": a16[c * BPC : (c + 1) * BPC],
            "b": b16[c * BPC : (c + 1) * BPC],
        }
        for c in range(N_CORES)
    ]
    res = run_bass_kernel_spmd(_get_nc(), in_maps, list(range(N_CORES)), trace=trace)
    out = np.concatenate([res.results[c]["out"] for c in range(N_CORES)], axis=0)
    return out, res


def kernel(a: np.ndarray, b: np.ndarray) -> np.ndarray:
    out, _ = run(np.asarray(a), np.asarray(b))
    return out


# revision 16
# speedup vs baseline: 1.0015x; 1.0006x over previous
"""Batched int8 GEMM (s8t x s8n -> s32t) on 8 TRN2 NeuronCores.

out[b, m, n] = sum_k a[b, m, k] * b[b, n, k]   (int32 accumulation)
a: [32, 1024, 1024] int8, b: [32, 1024, 1024] int8 -> out: [32, 1024, 1024] int32

Strategy:
  - Pure batch parallelism: 4 batches per core across 8 cores.
  - Both operands have K innermost, but the PE needs K on partitions.
    DMA-transpose works on 2-byte elements only, so we view the int8
    inputs as uint16 (pairs of adjacent K values) and DMA-transpose
    per-batch K-blocks of 256 K-values ([1024, 128] uint16 ->
    [128, 1024]), each partition holding an even/odd K pair interleaved
    along the free dim. Per-batch (rather than batch-pair) transposes
    keep each transpose's consumers within one batch so its DMA
    semaphore lane recycles quickly (8 HWDGE lanes rotate over all
    HWDGE DMAs; a lane is not reusable until the prior user's consumers
    have executed).
  - DVE deinterleaves (stride-2 int8 reads) and converts int8 -> bf16.
    int8 is exactly representable in bf16; products <= 2^14 and sums
    <= 2^24 are exact in fp32 PSUM accumulation, so the GEMM is
    bit-exact (native int8 matmul is rejected by walrus's BIR verifier,
    so bf16 is the fastest exact path; fp8 DoubleRow needs a 3-product
    Karatsuba digit split = 1.5x the PE cycles of bf16, a net loss).
  - PE: bf16 matmuls, K=128 per instruction, 8-step accumulation into
    [128, 512] fp32 PSUM banks (8 banks in flight). 16 dummy matmuls up
    front warm the HAM clock gate.
  - PSUM-freeing copies (fp32 PSUM -> int32 SBUF, exact) run on ACT
    (GPSIMD cannot access PSUM; DVE is busy with the deint stream).
    The final mt block's second copy goes on the by-then-idle DVE.
  - Stores: batches 0-1 issue one deferred 4MiB store each on ACT
    (deferred into the middle of the next batch's copy stream so the
    store's semaphore-lane WAR wait resolves during ACT idle time).
    Batch 2's 4MiB store is issued from SYNC (idle once all transposes
    are done) the moment batch 2's copies complete (~91us), so the
    output DMA queue is fully drained before the last batch's stores
    arrive. The last batch stores per-mt from SYNC (prompt dispatch
    from an otherwise-empty FIFO; on ACT they dispatch up to 2.2us
    late behind the next copy's semaphore wait), and the final mt
    block is split into two 256KB per-nt stores issued in parallel
    from ACT and SYNC so the kernel tail only waits on ~256KB.
"""

import numpy as np

import concourse.bass as bass
import concourse.mybir as mybir
import concourse.tile as tile
from concourse import bacc
from concourse.bass_utils import run_bass_kernel_spmd
from concourse.tile_rust import add_dep_helper

B, M, N, K = 32, 1024, 1024, 1024
N_CORES = 8
BPC = B // N_CORES  # batches per core
KB = K // 256  # k-blocks of 256 K-values (128 uint16 partitions)
N_TILE = 512
M_TILE = 128

_nc_cache = None


def build_nc():
    nc = bacc.Bacc("TRN2")

    # int8 inputs viewed as uint16 so the xbar DMA-transpose (2-byte
    # granularity) can be used straight out of HBM.
    a_in = nc.dram_tensor("a", [BPC, M, K // 2], mybir.dt.uint16, kind="ExternalInput")
    b_in = nc.dram_tensor("b", [BPC, N, K // 2], mybir.dt.uint16, kind="ExternalInput")
    out = nc.dram_tensor("out", [BPC, M, N], mybir.dt.int32, kind="ExternalOutput")

    with tile.TileContext(nc) as tc:
        with (
            tc.tile_pool(name="stage", bufs=2) as stage_pool,
            tc.tile_pool(name="conv", bufs=2) as conv_pool,
            tc.tile_pool(name="psum", bufs=8, space="PSUM") as psum_pool,
            tc.tile_pool(name="outbuf", bufs=2) as out_pool,
            tc.tile_pool(name="warm", bufs=1) as warm_pool,
            tc.tile_pool(name="stage_h", bufs=1) as stageh_pool,
            tc.tile_pool(name="conv_h", bufs=1) as convh_pool,
        ):
            # PE warmup: dummy matmuls with NO deps at all (uninitialized
            # SBUF reads are fine; the PSUM result is discarded), so the
            # HAM clock gate ramps before the real MM stream starts.
            # Same-bank start/stop matmuls pace at ~427ns each; 13 of them
            # bridge the ~5.6us between the PE becoming ready (~7.5us)
            # and the first full kt-pair of deinted tiles (~13us).
            wsrc = warm_pool.tile([128, N_TILE], mybir.dt.bfloat16, name="wsrc")
            nc.vector.memset(wsrc[:, :8], 0.0)
            wps = psum_pool.tile([128, N_TILE], mybir.dt.float32, name="wps", tag="ps")
            for _ in range(13):
                nc.tensor.matmul(wps[:], wsrc[:, :128], wsrc[:], start=True, stop=True)

            # Batches 0-1: stores are deferred into the middle of the NEXT
            # batch's copy stream (see docstring). Batch 2's store goes on
            # SYNC as soon as its data is ready; the last batch stores
            # per-mt on SYNC.
            pending_store = None
            ot_by_batch = {}
            HKB = 1  # kb blocks staged as M/N halves for batch 0
            for bi in range(BPC):
                n_kt = 2 * KB
                n_mt = M // M_TILE
                if bi == 0:
                    # ---- Batch 0: latency-critical prologue. Transpose
                    # desc-gen costs ~1.3us per instruction almost
                    # regardless of size (all on SYNC: issuing from two
                    # engines concurrently corrupts data - the 8 HWDGE
                    # semaphore lanes race across engines). kb0 is staged
                    # as [512, 128] M/N-halves so the first deinted tiles
                    # arrive ~1.5us sooner; a's M-upper half is only needed
                    # by the second mt-group (~25us in) and transposes
                    # last. ----
                    h_st = {}

                    def h_transpose(t, kb, h):
                        src = a_in if t == "a" else b_in
                        ht = stageh_pool.tile(
                            [128, 512],
                            mybir.dt.uint16,
                            name=f"h{t}{kb}{h}",
                            tag=f"h{t}{kb}{h}",
                        )
                        nc.sync.dma_start_transpose(
                            ht[:],
                            src[bi, h * 512 : (h + 1) * 512, kb * 128 : (kb + 1) * 128],
                        )
                        h_st[(t, kb, h)] = ht.bitcast(mybir.dt.int8)

                    for kb in range(HKB):
                        h_transpose("a", kb, 0)
                        h_transpose("b", kb, 0)
                        h_transpose("b", kb, 1)
                    a_st = {}
                    b_st = {}
                    for kb in range(HKB, KB):
                        at = stage_pool.tile(
                            [128, M], mybir.dt.uint16, name=f"at_{bi}_{kb}", tag=f"at{kb}"
                        )
                        nc.sync.dma_start_transpose(
                            at[:], a_in[bi, :, kb * 128 : (kb + 1) * 128]
                        )
                        a_st[kb] = at.bitcast(mybir.dt.int8)
                        bt = stage_pool.tile(
                            [128, N], mybir.dt.uint16, name=f"bt_{bi}_{kb}", tag=f"bt{kb}"
                        )
                        nc.sync.dma_start_transpose(
                            bt[:], b_in[bi, :, kb * 128 : (kb + 1) * 128]
                        )
                        b_st[kb] = bt.bitcast(mybir.dt.int8)
                    for kb in range(HKB):
                        h_transpose("a", kb, 1)

                    # ---- deints (DVE) in kt-consumption order: per kt the
                    # g0 stream needs a's M-lower half plus b's full N. ----
                    h_bf = {}

                    def h_deint(t, kb, h, par):
                        hb = convh_pool.tile(
                            [128, 512],
                            mybir.dt.bfloat16,
                            name=f"hb{t}{kb}{h}{par}",
                            tag=f"hb{t}{kb}{h}{par}",
                        )
                        nc.vector.tensor_copy(hb[:], h_st[(t, kb, h)][:, par::2])
                        h_bf[(t, kb, h, par)] = hb

                    a_bf = {}
                    b_bf = {}
                    for kb in range(HKB):
                        for par in range(2):
                            h_deint("a", kb, 0, par)
                            h_deint("b", kb, 0, par)
                            h_deint("b", kb, 1, par)
                    for kb in range(HKB, KB):
                        for par in range(2):
                            abf = conv_pool.tile(
                                [128, M],
                                mybir.dt.bfloat16,
                                name=f"abf_{bi}_{kb}_{par}",
                                tag=f"abf{kb}{par}",
                            )
                            nc.vector.tensor_copy(abf[:], a_st[kb][:, par::2])
                            a_bf[2 * kb + par] = abf
                            bbf = conv_pool.tile(
                                [128, N],
                                mybir.dt.bfloat16,
                                name=f"bbf_{bi}_{kb}_{par}",
                                tag=f"bbf{kb}{par}",
                            )
                            nc.vector.tensor_copy(bbf[:], b_st[kb][:, par::2])
                            b_bf[2 * kb + par] = bbf
                    for kb in range(HKB):
                        for par in range(2):
                            h_deint("a", kb, 1, par)

                    def a_sl(g, kt, mt):
                        kb, par = kt // 2, kt % 2
                        if kb < HKB:
                            return h_bf[("a", kb, g, par)][
                                :, (mt - 4 * g) * M_TILE : (mt - 4 * g + 1) * M_TILE
                            ]
                        return a_bf[kt][:, mt * M_TILE : (mt + 1) * M_TILE]

                    def b_sl(kt, nt):
                        kb, par = kt // 2, kt % 2
                        if kb < HKB:
                            return h_bf[("b", kb, nt, par)][:]
                        return b_bf[kt][:, nt * N_TILE : (nt + 1) * N_TILE]

                    ot = out_pool.tile(
                        [128, n_mt, N], mybir.dt.int32, name=f"ot_{bi}", tag="ot"
                    )
                    ot_by_batch[bi] = ot
                    # kt-outer over groups of 4 mt blocks (8 PSUM banks) so
                    # each arriving k-tile feeds 1.73us of real PE work; nt
                    # before mt inside a kt because b's N-upper half arrives
                    # ~0.35us after the N-lower half.
                    for g in range(n_mt // 4):
                        mts = range(4 * g, 4 * g + 4)
                        ps = {
                            (mt, nt): psum_pool.tile(
                                [128, N_TILE],
                                mybir.dt.float32,
                                name=f"ps_{bi}_{mt}_{nt}",
                                tag="ps",
                            )
                            for mt in mts
                            for nt in range(N // N_TILE)
                        }
                        for kt in range(n_kt):
                            for nt in range(N // N_TILE):
                                for mt in mts:
                                    nc.tensor.matmul(
                                        ps[(mt, nt)][:],
                                        a_sl(g, kt, mt),
                                        b_sl(kt, nt),
                                        start=(kt == 0),
                                        stop=(kt == n_kt - 1),
                                    )
                        for mt in mts:
                            for nt in range(N // N_TILE):
                                nc.scalar.copy(
                                    ot[:, mt, nt * N_TILE : (nt + 1) * N_TILE],
                                    ps[(mt, nt)][:],
                                )
                    pending_store = (bi, ot)
                    continue

                # ---- Batches 1-3: per-batch DMA-transpose staging: each
                # transpose's consumers (2 deints) execute within this
                # batch's prologue, so its semaphore lane recycles quickly. ----
                a_st = []
                b_st = []
                for kb in range(KB):
                    at = stage_pool.tile(
                        [128, M], mybir.dt.uint16, name=f"at_{bi}_{kb}", tag=f"at{kb}"
                    )
                    nc.sync.dma_start_transpose(at[:], a_in[bi, :, kb * 128 : (kb + 1) * 128])
                    a_st.append(at.bitcast(mybir.dt.int8))
                    bt = stage_pool.tile(
                        [128, N], mybir.dt.uint16, name=f"bt_{bi}_{kb}", tag=f"bt{kb}"
                    )
                    nc.sync.dma_start_transpose(bt[:], b_in[bi, :, kb * 128 : (kb + 1) * 128])
                    b_st.append(bt.bitcast(mybir.dt.int8))

                if bi == BPC - 1:
                    # Batch 2's 4MiB store, emitted right after the last
                    # batch's transposes: SYNC's FIFO is otherwise empty
                    # from ~60us on, so this dispatches the moment batch
                    # 2's copies complete (~91us) and the output queue is
                    # drained well before the last batch's stores arrive.
                    pbi = BPC - 2
                    nc.sync.dma_start(
                        out[pbi].rearrange("(t p) n -> p t n", p=128),
                        ot_by_batch[pbi][:],
                    )

                # ---- deinterleave + int8 -> bf16 (DVE) ----
                a_bf = []  # 8 bf16 tiles [128, M]; k-tile = kb*2+parity
                b_bf = []
                for kb in range(KB):
                    for par in range(2):
                        abf = conv_pool.tile(
                            [128, M],
                            mybir.dt.bfloat16,
                            name=f"abf_{bi}_{kb}_{par}",
                            tag=f"abf{kb}{par}",
                        )
                        nc.vector.tensor_copy(abf[:], a_st[kb][:, par::2])
                        a_bf.append(abf)
                        bbf = conv_pool.tile(
                            [128, N],
                            mybir.dt.bfloat16,
                            name=f"bbf_{bi}_{kb}_{par}",
                            tag=f"bbf{kb}{par}",
                        )
                        nc.vector.tensor_copy(bbf[:], b_st[kb][:, par::2])
                        b_bf.append(bbf)

                # ---- GEMM, accumulating in PSUM over kt. All 8 mt blocks
                # copy into one big staging tile. ----
                ot = out_pool.tile(
                    [128, n_mt, N], mybir.dt.int32, name=f"ot_{bi}", tag="ot"
                )
                ot_by_batch[bi] = ot
                if True:
                    # Steady-state batches: mt-outer so the PSUM-freeing
                    # copies spread evenly instead of bunching.
                    for mt in range(n_mt):
                        ps = [
                            psum_pool.tile(
                                [128, N_TILE],
                                mybir.dt.float32,
                                name=f"ps_{bi}_{mt}_{nt}",
                                tag="ps",
                            )
                            for nt in range(N // N_TILE)
                        ]
                        for kt in range(n_kt):
                            lhsT = a_bf[kt][:, mt * M_TILE : (mt + 1) * M_TILE]
                            for nt in range(N // N_TILE):
                                nc.tensor.matmul(
                                    ps[nt][:],
                                    lhsT,
                                    b_bf[kt][:, nt * N_TILE : (nt + 1) * N_TILE],
                                    start=(kt == 0),
                                    stop=(kt == n_kt - 1),
                                )
                        # fp32 -> int32 PSUM-freeing copies on ACT (exact:
                        # values are integers). For the very last mt block,
                        # the second copy goes on the (by then idle) DVE so
                        # the two copies run in parallel and the final
                        # stores start ~0.7us sooner.
                        if bi == BPC - 1 and mt == n_mt - 1:
                            act_copy = nc.scalar.copy(ot[:, mt, :N_TILE], ps[0][:])
                            nc.vector.tensor_copy(ot[:, mt, N_TILE:], ps[1][:])
                        else:
                            act_copy = None
                            for nt in range(N // N_TILE):
                                act_copy = nc.scalar.copy(
                                    ot[:, mt, nt * N_TILE : (nt + 1) * N_TILE], ps[nt][:]
                                )
                        if mt == 3 and pending_store is not None:
                            pbi, pot = pending_store
                            st = nc.scalar.dma_start(
                                out[pbi].rearrange("(t p) n -> p t n", p=128), pot[:]
                            )
                            # Ordering-only edge: keep the store (and its
                            # semaphore-lane WAR wait) behind this batch's
                            # mt0-3 ACT copies in the ACT FIFO.
                            add_dep_helper(
                                st.ins,
                                act_copy.ins,
                                False,
                                "defer batch store past next batch's early copies",
                            )
                            pending_store = None
                        if bi == BPC - 1:
                            if mt == n_mt - 1:
                                # Final mt: two 256KB per-nt stores issued in
                                # parallel from ACT and SYNC (store desc-gen
                                # is ~0.6us per instruction regardless of
                                # size, so finer splits only add latency).
                                nc.scalar.dma_start(
                                    out[bi, mt * M_TILE : (mt + 1) * M_TILE, :N_TILE],
                                    ot[:, mt, :N_TILE],
                                )
                                nc.sync.dma_start(
                                    out[bi, mt * M_TILE : (mt + 1) * M_TILE, N_TILE:],
                                    ot[:, mt, N_TILE:],
                                )
                            else:
                                # Last batch: store per mt from SYNC (prompt
                                # dispatch; on ACT these sit up to 2.2us
                                # behind the next copy's semaphore wait).
                                nc.sync.dma_start(
                                    out[bi, mt * M_TILE : (mt + 1) * M_TILE, :],
                                    ot[:, mt, :],
                                )
                if bi < BPC - 2:
                    # Batches 0-1: one 4MiB store for the whole batch,
                    # deferred (emitted mid-next-batch, see above). HBM row
                    # (mt*128 + p) pairs with SBUF [p, mt, :].
                    pending_store = (bi, ot)
    nc.compile()
    return nc


def _get_nc():
    global _nc_cache
    if _nc_cache is None:
        _nc_cache = build_nc()
    return _nc_cache


def run(a: np.ndarray, b: np.ndarray, trace: bool = False):
    """Run on 8 cores. a/b: [32, 1024, 1024] int8. Returns (out, BassKernelResults)."""
    a = np.ascontiguousarray(a)
    b = np.ascontiguousarray(b)
    a16 = a.view(np.uint16).reshape(B, M, K // 2)
    b16 = b.view(np.uint16).reshape(B, N, K // 2)
    in_maps = [
        {
            "a": a16[c * BPC : (c + 1) * BPC],
            "b": b16[c * BPC : (c + 1) * BPC],
        }
        for c in range(N_CORES)
    ]
    res = run_bass_kernel_spmd(_get_nc(), in_maps, list(range(N_CORES)), trace=trace)
    out = np.concatenate([res.results[c]["out"] for c in range(N_CORES)], axis=0)
    return out, res


def kernel(a: np.ndarray, b: np.ndarray) -> np.ndarray:
    out, _ = run(np.asarray(a), np.asarray(b))
    return out


# revision 17
# speedup vs baseline: 1.0076x; 1.0061x over previous
"""Batched int8 GEMM (s8t x s8n -> s32t) on 8 TRN2 NeuronCores.

out[b, m, n] = sum_k a[b, m, k] * b[b, n, k]   (int32 accumulation)
a: [32, 1024, 1024] int8, b: [32, 1024, 1024] int8 -> out: [32, 1024, 1024] int32

Strategy:
  - Pure batch parallelism: 4 batches per core across 8 cores.
  - Both operands have K innermost, but the PE needs K on partitions.
    DMA-transpose works on 2-byte elements only, so we view the int8
    inputs as uint16 (pairs of adjacent K values) and DMA-transpose
    per-batch K-blocks of 256 K-values ([1024, 128] uint16 ->
    [128, 1024]), each partition holding an even/odd K pair interleaved
    along the free dim. Per-batch (rather than batch-pair) transposes
    keep each transpose's consumers within one batch so its DMA
    semaphore lane recycles quickly (8 HWDGE lanes rotate over all
    HWDGE DMAs; a lane is not reusable until the prior user's consumers
    have executed).
  - DVE deinterleaves (stride-2 int8 reads) and converts int8 -> bf16.
    int8 is exactly representable in bf16; products <= 2^14 and sums
    <= 2^24 are exact in fp32 PSUM accumulation, so the GEMM is
    bit-exact (native int8 matmul is rejected by walrus's BIR verifier,
    so bf16 is the fastest exact path; fp8 DoubleRow needs a 3-product
    Karatsuba digit split = 1.5x the PE cycles of bf16, a net loss).
  - PE: bf16 matmuls, K=128 per instruction, 8-step accumulation into
    [128, 512] fp32 PSUM banks (8 banks in flight). 16 dummy matmuls up
    front warm the HAM clock gate.
  - PSUM-freeing copies (fp32 PSUM -> int32 SBUF, exact) run on ACT
    (GPSIMD cannot access PSUM; DVE is busy with the deint stream).
    The final mt block's second copy goes on the by-then-idle DVE.
  - Stores: batches 0-1 issue one deferred 4MiB store each on ACT
    (deferred into the middle of the next batch's copy stream so the
    store's semaphore-lane WAR wait resolves during ACT idle time).
    Batch 2's 4MiB store is issued from SYNC (idle once all transposes
    are done) the moment batch 2's copies complete (~91us), so the
    output DMA queue is fully drained before the last batch's stores
    arrive. The last batch stores per-mt from SYNC (prompt dispatch
    from an otherwise-empty FIFO; on ACT they dispatch up to 2.2us
    late behind the next copy's semaphore wait), and the final mt
    block is split into two 256KB per-nt stores issued in parallel
    from ACT and SYNC so the kernel tail only waits on ~256KB.
"""

import numpy as np

import concourse.bass as bass
import concourse.mybir as mybir
import concourse.tile as tile
from concourse import bacc
from concourse.bass_utils import run_bass_kernel_spmd
from concourse.tile_rust import add_dep_helper

B, M, N, K = 32, 1024, 1024, 1024
N_CORES = 8
BPC = B // N_CORES  # batches per core
KB = K // 256  # k-blocks of 256 K-values (128 uint16 partitions)
N_TILE = 512
M_TILE = 128

_nc_cache = None


def build_nc():
    nc = bacc.Bacc("TRN2")

    # int8 inputs viewed as uint16 so the xbar DMA-transpose (2-byte
    # granularity) can be used straight out of HBM.
    a_in = nc.dram_tensor("a", [BPC, M, K // 2], mybir.dt.uint16, kind="ExternalInput")
    b_in = nc.dram_tensor("b", [BPC, N, K // 2], mybir.dt.uint16, kind="ExternalInput")
    out = nc.dram_tensor("out", [BPC, M, N], mybir.dt.int32, kind="ExternalOutput")

    with tile.TileContext(nc) as tc:
        with (
            tc.tile_pool(name="stage", bufs=2) as stage_pool,
            tc.tile_pool(name="conv", bufs=2) as conv_pool,
            tc.tile_pool(name="psum", bufs=8, space="PSUM") as psum_pool,
            tc.tile_pool(name="outbuf", bufs=2) as out_pool,
            tc.tile_pool(name="warm", bufs=1) as warm_pool,
        ):
            # PE warmup: dummy matmuls with NO deps at all (uninitialized
            # SBUF reads are fine; the PSUM result is discarded), so the
            # HAM clock gate ramps before the real MM stream starts.
            # Same-bank start/stop matmuls pace at ~427ns each; 13 of them
            # bridge the ~5.6us between the PE becoming ready (~7.5us)
            # and the first full kt-pair of deinted tiles (~13us).
            wsrc = warm_pool.tile([128, N_TILE], mybir.dt.bfloat16, name="wsrc")
            nc.vector.memset(wsrc[:, :8], 0.0)
            wps = psum_pool.tile([128, N_TILE], mybir.dt.float32, name="wps", tag="ps")
            for _ in range(13):
                nc.tensor.matmul(wps[:], wsrc[:, :128], wsrc[:], start=True, stop=True)

            # Batches 0-1: stores are deferred into the middle of the NEXT
            # batch's copy stream (see docstring). Batch 2's store goes on
            # SYNC as soon as its data is ready; the last batch stores
            # per-mt on SYNC.
            pending_store = None
            ot_by_batch = {}
            for bi in range(BPC):
                n_kt = 2 * KB
                n_mt = M // M_TILE
                if bi == 0:
                    # ---- Batch 0: its transpose+deint prologue is the
                    # latency-critical path to the first real matmul.
                    # Transpose desc-gen costs ~1.3us per instruction almost
                    # regardless of size and must stay on ONE engine
                    # (issuing from two engines concurrently corrupts data -
                    # the 8 HWDGE semaphore lanes race across engines), so
                    # the serial chain a0,b0,a1,b1,... is the supply bound:
                    # the first kt-pair is deinted by ~13us and later kt
                    # arrivals just keep ahead of the kt-outer GEMM below. ----
                    a_st = []
                    b_st = []
                    for kb in range(KB):
                        at = stage_pool.tile(
                            [128, M], mybir.dt.uint16, name=f"at_{bi}_{kb}", tag=f"at{kb}"
                        )
                        nc.sync.dma_start_transpose(
                            at[:], a_in[bi, :, kb * 128 : (kb + 1) * 128]
                        )
                        a_st.append(at.bitcast(mybir.dt.int8))
                        bt = stage_pool.tile(
                            [128, N], mybir.dt.uint16, name=f"bt_{bi}_{kb}", tag=f"bt{kb}"
                        )
                        nc.sync.dma_start_transpose(
                            bt[:], b_in[bi, :, kb * 128 : (kb + 1) * 128]
                        )
                        b_st.append(bt.bitcast(mybir.dt.int8))

                    # deints (DVE) in kt-consumption order
                    a_bf = []
                    b_bf = []
                    for kb in range(KB):
                        for par in range(2):
                            abf = conv_pool.tile(
                                [128, M],
                                mybir.dt.bfloat16,
                                name=f"abf_{bi}_{kb}_{par}",
                                tag=f"abf{kb}{par}",
                            )
                            nc.vector.tensor_copy(abf[:], a_st[kb][:, par::2])
                            a_bf.append(abf)
                            bbf = conv_pool.tile(
                                [128, N],
                                mybir.dt.bfloat16,
                                name=f"bbf_{bi}_{kb}_{par}",
                                tag=f"bbf{kb}{par}",
                            )
                            nc.vector.tensor_copy(bbf[:], b_st[kb][:, par::2])
                            b_bf.append(bbf)

                    ot = out_pool.tile(
                        [128, n_mt, N], mybir.dt.int32, name=f"ot_{bi}", tag="ot"
                    )
                    ot_by_batch[bi] = ot
                    # kt-outer over groups of 4 mt blocks (8 PSUM banks) so
                    # each arriving k-tile feeds 1.73us of real PE work and
                    # the ramp is gapless.
                    for g in range(n_mt // 4):
                        mts = range(4 * g, 4 * g + 4)
                        ps = {
                            (mt, nt): psum_pool.tile(
                                [128, N_TILE],
                                mybir.dt.float32,
                                name=f"ps_{bi}_{mt}_{nt}",
                                tag="ps",
                            )
                            for mt in mts
                            for nt in range(N // N_TILE)
                        }
                        for kt in range(n_kt):
                            for mt in mts:
                                lhsT = a_bf[kt][:, mt * M_TILE : (mt + 1) * M_TILE]
                                for nt in range(N // N_TILE):
                                    nc.tensor.matmul(
                                        ps[(mt, nt)][:],
                                        lhsT,
                                        b_bf[kt][:, nt * N_TILE : (nt + 1) * N_TILE],
                                        start=(kt == 0),
                                        stop=(kt == n_kt - 1),
                                    )
                        for mt in mts:
                            for nt in range(N // N_TILE):
                                nc.scalar.copy(
                                    ot[:, mt, nt * N_TILE : (nt + 1) * N_TILE],
                                    ps[(mt, nt)][:],
                                )
                    pending_store = (bi, ot)
                    continue

                # ---- Batches 1-3: per-batch DMA-transpose staging: each
                # transpose's consumers (2 deints) execute within this
                # batch's prologue, so its semaphore lane recycles quickly. ----
                a_st = []
                b_st = []
                for kb in range(KB):
                    at = stage_pool.tile(
                        [128, M], mybir.dt.uint16, name=f"at_{bi}_{kb}", tag=f"at{kb}"
                    )
                    nc.sync.dma_start_transpose(at[:], a_in[bi, :, kb * 128 : (kb + 1) * 128])
                    a_st.append(at.bitcast(mybir.dt.int8))
                    bt = stage_pool.tile(
                        [128, N], mybir.dt.uint16, name=f"bt_{bi}_{kb}", tag=f"bt{kb}"
                    )
                    nc.sync.dma_start_transpose(bt[:], b_in[bi, :, kb * 128 : (kb + 1) * 128])
                    b_st.append(bt.bitcast(mybir.dt.int8))

                if bi == BPC - 1:
                    # Batch 2's 4MiB store, emitted right after the last
                    # batch's transposes: SYNC's FIFO is otherwise empty
                    # from ~60us on, so this dispatches the moment batch
                    # 2's copies complete (~91us) and the output queue is
                    # drained well before the last batch's stores arrive.
                    pbi = BPC - 2
                    nc.sync.dma_start(
                        out[pbi].rearrange("(t p) n -> p t n", p=128),
                        ot_by_batch[pbi][:],
                    )

                # ---- deinterleave + int8 -> bf16 (DVE) ----
                a_bf = []  # 8 bf16 tiles [128, M]; k-tile = kb*2+parity
                b_bf = []
                for kb in range(KB):
                    for par in range(2):
                        abf = conv_pool.tile(
                            [128, M],
                            mybir.dt.bfloat16,
                            name=f"abf_{bi}_{kb}_{par}",
                            tag=f"abf{kb}{par}",
                        )
                        nc.vector.tensor_copy(abf[:], a_st[kb][:, par::2])
                        a_bf.append(abf)
                        bbf = conv_pool.tile(
                            [128, N],
                            mybir.dt.bfloat16,
                            name=f"bbf_{bi}_{kb}_{par}",
                            tag=f"bbf{kb}{par}",
                        )
                        nc.vector.tensor_copy(bbf[:], b_st[kb][:, par::2])
                        b_bf.append(bbf)

                # ---- GEMM, accumulating in PSUM over kt. All 8 mt blocks
                # copy into one big staging tile. ----
                ot = out_pool.tile(
                    [128, n_mt, N], mybir.dt.int32, name=f"ot_{bi}", tag="ot"
                )
                ot_by_batch[bi] = ot
                if True:
                    # Steady-state batches: mt-outer so the PSUM-freeing
                    # copies spread evenly instead of bunching.
                    for mt in range(n_mt):
                        ps = [
                            psum_pool.tile(
                                [128, N_TILE],
                                mybir.dt.float32,
                                name=f"ps_{bi}_{mt}_{nt}",
                                tag="ps",
                            )
                            for nt in range(N // N_TILE)
                        ]
                        for kt in range(n_kt):
                            lhsT = a_bf[kt][:, mt * M_TILE : (mt + 1) * M_TILE]
                            for nt in range(N // N_TILE):
                                nc.tensor.matmul(
                                    ps[nt][:],
                                    lhsT,
                                    b_bf[kt][:, nt * N_TILE : (nt + 1) * N_TILE],
                                    start=(kt == 0),
                                    stop=(kt == n_kt - 1),
                                )
                        # fp32 -> int32 PSUM-freeing copies on ACT (exact:
                        # values are integers). For the very last mt block,
                        # the second copy goes on the (by then idle) DVE so
                        # the two copies run in parallel and the final
                        # stores start ~0.7us sooner.
                        if bi == BPC - 1 and mt == n_mt - 1:
                            act_copy = nc.scalar.copy(ot[:, mt, :N_TILE], ps[0][:])
                            nc.vector.tensor_copy(ot[:, mt, N_TILE:], ps[1][:])
                        else:
                            act_copy = None
                            for nt in range(N // N_TILE):
                                act_copy = nc.scalar.copy(
                                    ot[:, mt, nt * N_TILE : (nt + 1) * N_TILE], ps[nt][:]
                                )
                        if mt == 3 and pending_store is not None:
                            pbi, pot = pending_store
                            st = nc.scalar.dma_start(
                                out[pbi].rearrange("(t p) n -> p t n", p=128), pot[:]
                            )
                            # Ordering-only edge: keep the store (and its
                            # semaphore-lane WAR wait) behind this batch's
                            # mt0-3 ACT copies in the ACT FIFO.
                            add_dep_helper(
                                st.ins,
                                act_copy.ins,
                                False,
                                "defer batch store past next batch's early copies",
                            )
                            pending_store = None
                        if bi == BPC - 1:
                            if mt == n_mt - 1:
                                # Final mt: two 256KB per-nt stores issued in
                                # parallel from ACT and SYNC (store desc-gen
                                # is ~0.6us per instruction regardless of
                                # size, so finer splits only add latency).
                                nc.scalar.dma_start(
                                    out[bi, mt * M_TILE : (mt + 1) * M_TILE, :N_TILE],
                                    ot[:, mt, :N_TILE],
                                )
                                nc.sync.dma_start(
                                    out[bi, mt * M_TILE : (mt + 1) * M_TILE, N_TILE:],
                                    ot[:, mt, N_TILE:],
                                )
                            else:
                                # Last batch: store per mt from SYNC (prompt
                                # dispatch; on ACT these sit up to 2.2us
                                # behind the next copy's semaphore wait).
                                nc.sync.dma_start(
                                    out[bi, mt * M_TILE : (mt + 1) * M_TILE, :],
                                    ot[:, mt, :],
                                )
                if bi < BPC - 2:
                    # Batches 0-1: one 4MiB store for the whole batch,
                    # deferred (emitted mid-next-batch, see above). HBM row
                    # (mt*128 + p) pairs with SBUF [p, mt, :].
                    pending_store = (bi, ot)
    nc.compile()
    return nc


def _get_nc():
    global _nc_cache
    if _nc_cache is None:
        _nc_cache = build_nc()
    return _nc_cache


def run(a: np.ndarray, b: np.ndarray, trace: bool = False):
    """Run on 8 cores. a/b: [32, 1024, 1024] int8. Returns (out, BassKernelResults)."""
    a = np.ascontiguousarray(a)
    b = np.ascontiguousarray(b)
    a16 = a.view(np.uint16).reshape(B, M, K // 2)
    b16 = b.view(np.uint16).reshape(B, N, K // 2)
    in_maps = [
        {
            "a": a16[c * BPC : (c + 1) * BPC],
            "b": b16[c * BPC : (c + 1) * BPC],
        }
        for c in range(N_CORES)
    ]
    res = run_bass_kernel_spmd(_get_nc(), in_maps, list(range(N_CORES)), trace=trace)
    out = np.concatenate([res.results[c]["out"] for c in range(N_CORES)], axis=0)
    return out, res


def kernel(a: np.ndarray, b: np.ndarray) -> np.ndarray:
    out, _ = run(np.asarray(a), np.asarray(b))
    return out
